# revision 5
# baseline (speedup 1.0000x reference)
"""Trainium2 Bass kernel for the GCN layer problem (8-core row-sharded SPMD).

Contract: kernel(**inputs) takes the FULL inputs of reference.setup_inputs()
and returns the FULL output tuple (out [8192,64] f32, A1 [8192,8192] f32).

Strategy
--------
Shard the N=8192 node dimension across the 8 NeuronCores (1024 rows each).
Host (cheap, tiny tensors): BatchNorm stats, Hx = Hn@W1+b1, HWs = Hn@Wout+bout,
the uint8 mask (A > 0), and a bf16 multi-term split of Hx (Hx = s0 + s1 (+ s2),
each bf16) so the score matmul runs at bf16 PE rate with near-fp32 accuracy
via product expansion (s0@s0 + s0@s1 + s1@s0 [+ ...]).

Device (per core - the memory-bound NxN part):
  logits = Hx_mine @ Hx_all^T            (TensorE, accumulating bf16 products)
  B      = max(sigmoid(logits), 0.1)     (ScalarE: relu-clamp + sigmoid,
                                          using max(sig(x),.1)=sig(max(x,-L)))
  A1blk  = B * mask, rowsum(A1blk)       (one fused VectorE op with accum_out)
  d      = rsqrt(rowsum + 1)             (Newton-refined)
  PT    += (d*HWs_mine)^T @ B            (TensorE, contracts over own rows)

Because B is symmetric (Hx@Hx.T is), sum over cores of PT equals
(B @ (d*HWs))^T - no transposes and no second pass over the NxN data; the only
cross-core combination is an 8-way sum of the small [64, N] PT tensors, done
on the host. Host combine: out = leaky(d * (sum(PT).T + d*HWs - corrections)),
where corrections handle the (expected ~8) exact zeros of A whose mask breaks
the symmetry, and the +I diagonal is added on the host.

Modes (env GCN_MODE): "fast" (default) - 2-term split, float32r partial
matmul (A1 ~2e-4 absmax, out ~1.3e-4 rel vs fp64). "exact" - 3-term split,
fp32 partial (A1/out ~3e-6), ~1.5x slower.
"""
import numpy as np

N = 8192
P = 8
ROWS = N // P
IN_DIM = 128
THETA = 256
OUT_DIM = 64
BN_EPS = 1e-5
SLOPE = 0.01
LOGIT0 = 2.1972245773362196  # -logit(0.1)

LAST_EXEC_NS = None  # filled when GCN_TRACE=1


def _import_concourse():
    import importlib
    try:
        importlib.import_module("concourse.bass")
    except ImportError:
        import sys
        for p in ("/opt/trn_rl_repo", "/root/.axon_site/_ro/trn_rl_repo"):
            if p not in sys.path:
                sys.path.append(p)


def _split_multi_waits(nc, mybir):
    """This container's walrus build rejects >1 sync-wait per engine
    instruction ("Too many sync wait commands"). Hoist extra waits onto
    same-engine InstNoOps placed immediately before the instruction."""
    n = [0]

    def fresh_name():
        n[0] += 1
        return f"waitsplit-{n[0]}"

    for f in nc.m.functions:
        for bb in f.blocks:
            out = []
            changed = False
            for inst in bb.instructions:
                si = inst.sync_info
                waits = list(si.on_wait) if si and si.on_wait else []
                if len(waits) > 1:
                    for w in waits[:-1]:
                        nop = mybir.InstNoOp(name=fresh_name(), ins=[], outs=[])
                        nop.engine = inst.engine
                        nop.sync_info = mybir.SyncInfo(on_wait=[w], on_update=[])
                        out.append(nop)
                    si.on_wait = [waits[-1]]
                    inst.sync_info = si
                    changed = True
                out.append(inst)
            if changed:
                bb.instructions = out


def _patch_tile_drain():
    """Same walrus limitation hits the TileContext exit drain: move its waits
    onto single-wait SP nops."""
    import concourse.tile as tile
    from concourse import mybir
    from concourse.vector_clock import ScopedClock

    if getattr(tile.TileContext, "_gcn_drain_patched", False):
        return

    def _drain_and_barrier(self, tick_clock, wait_clock):
        nc = self.nc
        nop_inst = nc.sync.nop(nofuse=True)
        wait_clock.add_sem_waits(
            nop_inst.ins, ScopedClock({None: tick_clock.global_clock}))
        si = nop_inst.ins.sync_info
        waits = list(si.on_wait) if si and si.on_wait else []
        if len(waits) > 1:
            si.on_wait = [waits[0]]
            nop_inst.ins.sync_info = si
            for w in waits[1:]:
                extra = nc.sync.nop(nofuse=True)
                esi = extra.ins.sync_info
                if esi is None:
                    esi = mybir.SyncInfo(on_wait=[w], on_update=[])
                else:
                    esi.on_wait = [w]
                extra.ins.sync_info = esi
        nc.sync.drain()
        nc.all_engine_barrier()
        assert self.sems is not None
        popped = nc._tile_sem_poison_stack.pop()
        assert popped is self._sem_poison
        nc.clear_and_free_semaphores(list(self.sems.allocated().values()))
        nc.all_engine_barrier()

    tile.TileContext._drain_and_barrier = _drain_and_barrier
    tile.TileContext._gcn_drain_patched = True


def _build_nc(mode="fast"):
    import concourse.bass as bass
    import concourse.tile as tile
    from contextlib import ExitStack
    from concourse import mybir

    F32 = mybir.dt.float32
    F32R = mybir.dt.float32r
    BF16 = mybir.dt.bfloat16
    U8 = mybir.dt.uint8
    AF = mybir.ActivationFunctionType
    ALU = mybir.AluOpType

    RC = ROWS // 128
    if mode == "fast":
        n_split = 2
        terms = [(0, 0), (0, 1), (1, 0)]
        partial_f32r, b_f32r = True, True
        wide, b_bufs = 2048, 2
    else:
        n_split = 3
        terms = [(0, 0), (0, 1), (1, 0), (0, 2), (2, 0), (1, 1)]
        partial_f32r, b_f32r = False, False
        wide, b_bufs = 1024, 1
    NW = N // wide
    NPW = wide // 512

    _patch_tile_drain()
    nc = bass.Bass()
    a_blk = nc.declare_dram_parameter("a_blk", [ROWS, N], U8, isOutput=False)
    hxs = [nc.declare_dram_parameter(f"hxs{i}", [THETA, N], BF16, isOutput=False)
           for i in range(n_split)]
    hxms = [nc.declare_dram_parameter(f"hxms{i}", [THETA, ROWS], BF16,
                                      isOutput=False) for i in range(n_split)]
    hws = nc.declare_dram_parameter("hws", [ROWS, OUT_DIM], F32, isOutput=False)
    a1_blk = nc.declare_dram_parameter("a1_blk", [ROWS, N], F32, isOutput=True)
    pt_out = nc.declare_dram_parameter("pt_out", [OUT_DIM, N], F32, isOutput=True)
    rs_out = nc.declare_dram_parameter("rs_out", [RC, 128], F32, isOutput=True)

    with ExitStack() as ctx:
        tc = ctx.enter_context(tile.TileContext(nc))
        const = ctx.enter_context(tc.tile_pool(name="const", bufs=1))
        bpool = ctx.enter_context(tc.tile_pool(name="b", bufs=b_bufs))
        apool = ctx.enter_context(tc.tile_pool(name="a", bufs=2))
        small = ctx.enter_context(tc.tile_pool(name="small", bufs=2))
        psum_s = ctx.enter_context(tc.tile_pool(name="ps_s", bufs=4, space="PSUM"))
        psum_p = ctx.enter_context(tc.tile_pool(name="ps_p", bufs=4, space="PSUM"))

        bias_pos = const.tile([128, 1], F32, tag="bias_pos")
        nc.vector.memset(bias_pos[:], LOGIT0)
        bias_neg = const.tile([128, 1], F32, tag="bias_neg")
        nc.vector.memset(bias_neg[:], -LOGIT0)

        sx = [[const.tile([128, N], BF16, tag=f"sx{i}_{t}", name=f"sx{i}_{t}")
               for t in range(2)] for i in range(n_split)]
        sm = [[const.tile([128, ROWS], BF16, tag=f"sm{i}_{t}", name=f"sm{i}_{t}")
               for t in range(2)] for i in range(n_split)]
        for i in range(n_split):
            for t in range(2):
                nc.sync.dma_start(sm[i][t][:], hxms[i][t * 128:(t + 1) * 128, :])
        # q-major so the first columns of every split land first and the PE
        # can start ~5us in
        qn = N // 8
        for q in range(8):
            for i in range(n_split):
                for t in range(2):
                    nc.sync.dma_start(
                        sx[i][t][:, q * qn:(q + 1) * qn],
                        hxs[i][t * 128:(t + 1) * 128, q * qn:(q + 1) * qn])
        hws_sb = const.tile([128, RC * OUT_DIM], F32, tag="hws")
        for rc in range(RC):
            nc.sync.dma_start(hws_sb[:, rc * OUT_DIM:(rc + 1) * OUT_DIM],
                              hws[rc * 128:(rc + 1) * 128, :])
        pt_acc = const.tile([OUT_DIM, N], F32, tag="pt_acc")

        def emit_scores(rc):
            r0 = rc * 128
            b = bpool.tile([128, N], F32, tag="b", name="b")
            rs_cols = small.tile([128, NW], F32, tag="rscols", name="rs_cols")
            for w in range(NW):
                w0 = w * wide
                a_t = apool.tile([128, wide], U8, tag="a_in", name="a_t")
                nc.sync.dma_start(a_t[:], a_blk[r0:r0 + 128, w0:w0 + wide])
                for pk in range(NPW):
                    c0 = w0 + pk * 512
                    ps = psum_s.tile([128, 512], F32, tag="ps", name="ps")
                    n_mm = 2 * len(terms)
                    k = 0
                    for t in range(2):
                        for (li, ri) in terms:
                            nc.tensor.matmul(
                                ps[:], sm[li][t][:, r0:r0 + 128],
                                sx[ri][t][:, c0:c0 + 512],
                                start=(k == 0), stop=(k == n_mm - 1))
                            k += 1
                    # max(sigmoid(x), 0.1) == sigmoid(relu(x + L) - L)
                    nc.scalar.activation(ps[:], ps[:], AF.Relu, bias=bias_pos[:])
                    sig_dst = b[:, c0:c0 + 512]
                    if b_f32r:
                        sig_dst = sig_dst.bitcast(F32R)
                    nc.scalar.activation(sig_dst, ps[:], AF.Sigmoid,
                                         bias=bias_neg[:])
                a1_t = apool.tile([128, wide], F32, tag="a1_out", name="a1_t")
                nc.vector.scalar_tensor_tensor(
                    out=a1_t[:], in0=a_t[:], scalar=0.0,
                    in1=b[:, w0:w0 + wide],
                    op0=ALU.is_gt, op1=ALU.mult,
                    accum_out=rs_cols[:, w:w + 1],
                )
                nc.gpsimd.dma_start(a1_blk[r0:r0 + 128, w0:w0 + wide], a1_t[:])
            return b, rs_cols

        def emit_partials(rc, b, rs_cols):
            rs = small.tile([128, 1], F32, tag="rs", name="rs")
            nc.vector.tensor_reduce(rs[:], rs_cols[:], axis=mybir.AxisListType.X,
                                    op=ALU.add)
            nc.sync.dma_start(rs_out[rc, :], rs[:, 0:1])
            sq = small.tile([128, 1], F32, tag="sq", name="sq")
            nc.scalar.activation(sq[:], rs[:], AF.Sqrt, bias=1.0)
            d0 = small.tile([128, 1], F32, tag="d0", name="d0")
            nc.vector.reciprocal(d0[:], sq[:])
            # Newton step for rsqrt accuracy: d1 = d0*(1.5 - 0.5*(rs+1)*d0^2)
            rsp1 = small.tile([128, 1], F32, tag="rsp1", name="rsp1")
            nc.vector.tensor_scalar_add(rsp1[:], rs[:], 1.0)
            t_dd = small.tile([128, 1], F32, tag="tdd", name="t_dd")
            nc.vector.tensor_mul(t_dd[:], d0[:], d0[:])
            t_xdd = small.tile([128, 1], F32, tag="txdd", name="t_xdd")
            nc.vector.tensor_mul(t_xdd[:], rsp1[:], t_dd[:])
            t_h = small.tile([128, 1], F32, tag="th", name="t_h")
            nc.vector.tensor_scalar(t_h[:], t_xdd[:], -0.5, 1.5,
                                    op0=ALU.mult, op1=ALU.add)
            d1 = small.tile([128, 1], F32, tag="d1", name="d1")
            nc.vector.tensor_mul(d1[:], t_h[:], d0[:])
            x_rc = small.tile([128, OUT_DIM], F32, tag="x", name="x_rc")
            x_dst = x_rc[:].bitcast(F32R) if partial_f32r else x_rc[:]
            nc.vector.tensor_scalar_mul(
                x_dst, hws_sb[:, rc * OUT_DIM:(rc + 1) * OUT_DIM], d1[:])

            lhs_x = x_rc[:].bitcast(F32R) if partial_f32r else x_rc[:]
            for ms in range(N // 512):
                m0 = ms * 512
                ps2 = psum_p.tile([OUT_DIM, 512], F32, tag="ps2", name="ps2")
                rhs_b = b[:, m0:m0 + 512]
                if partial_f32r:
                    rhs_b = rhs_b.bitcast(F32R)
                nc.tensor.matmul(ps2[:], lhs_x, rhs_b, start=True, stop=True)
                if rc == 0:
                    nc.vector.tensor_copy(pt_acc[:, m0:m0 + 512], ps2[:])
                else:
                    nc.vector.tensor_add(pt_acc[:, m0:m0 + 512],
                                         pt_acc[:, m0:m0 + 512], ps2[:])

        # software pipeline: partials of rc-1 run behind scores of rc (PE is
        # in-order and would otherwise stall on the DVE rowsum -> d chain)
        prev = None
        for rc in range(RC):
            cur = emit_scores(rc)
            if prev is not None:
                emit_partials(rc - 1, *prev)
            prev = cur
        emit_partials(RC - 1, *prev)

        ptq = N // 4
        for q in range(4):
            nc.sync.dma_start(pt_out[:, q * ptq:(q + 1) * ptq],
                              pt_acc[:, q * ptq:(q + 1) * ptq])

    _split_multi_waits(nc, mybir)
    return nc, n_split


_NC_CACHE = {}


def _get_nc(mode):
    if mode not in _NC_CACHE:
        _NC_CACHE[mode] = _build_nc(mode)
    return _NC_CACHE[mode]


def kernel(H, A, bn_gamma, bn_beta, W1, b1, Wout, bout):
    import os
    _import_concourse()
    import ml_dtypes
    from concourse.bass_utils import run_bass_kernel_spmd

    H = np.asarray(H, np.float32)
    A = np.asarray(A, np.float32)

    # Host precompute of the small node-feature tensors
    mean = H.mean(0, dtype=np.float64)
    var = np.square(H - mean).mean(0, dtype=np.float64)
    Hn = ((H - mean) / np.sqrt(var + BN_EPS) * np.asarray(bn_gamma, np.float64)
          + np.asarray(bn_beta, np.float64)).astype(np.float32)
    Hx = (Hn @ np.asarray(W1, np.float32)
          + np.asarray(b1, np.float32)).astype(np.float32)
    HWs = (Hn @ np.asarray(Wout, np.float32)
           + np.asarray(bout, np.float32)).astype(np.float32)

    mode = os.environ.get("GCN_MODE", "fast")
    nc, n_split = _get_nc(mode)

    # bf16 multi-term split of HxT and the uint8 adjacency mask
    HxT = np.ascontiguousarray(Hx.T)
    splits = []
    resid = HxT
    for _ in range(n_split):
        s = resid.astype(ml_dtypes.bfloat16)
        resid = resid - s.astype(np.float32)
        splits.append(s)
    mask8 = (A > 0).astype(np.uint8)

    in_maps = []
    for i in range(P):
        sl = slice(i * ROWS, (i + 1) * ROWS)
        m = {"a_blk": np.ascontiguousarray(mask8[sl]),
             "hws": np.ascontiguousarray(HWs[sl])}
        for j, s in enumerate(splits):
            m[f"hxs{j}"] = s
            m[f"hxms{j}"] = np.ascontiguousarray(s[:, sl])
        in_maps.append(m)

    trace = os.environ.get("GCN_TRACE", "0") == "1"
    try:
        res = run_bass_kernel_spmd(
            nc, in_maps, list(range(P)), trace=trace,
            trace_cores=list(range(P)) if trace else None)
    except Exception:
        # transient NRT flake (e.g. NRT_EXEC_UNIT_UNRECOVERABLE): retry once
        res = run_bass_kernel_spmd(
            nc, in_maps, list(range(P)), trace=trace,
            trace_cores=list(range(P)) if trace else None)
    global LAST_EXEC_NS
    LAST_EXEC_NS = res.exec_time_ns

    # Host combine
    A1 = np.concatenate([r["a1_blk"] for r in res.results], 0)
    idx = np.arange(N)
    A1[idx, idx] += 1.0
    rowsums = np.concatenate(
        [r["rs_out"].reshape(-1) for r in res.results]) + 1.0
    PT = np.zeros((OUT_DIM, N), np.float64)
    for r in res.results:
        PT += r["pt_out"]
    d = rowsums.astype(np.float64) ** -0.5
    HWs64 = HWs.astype(np.float64)
    pre = PT.T + d[:, None] * HWs64
    # corrections for exact zeros of A (mask breaks the symmetry of B there)
    zr, zc = np.nonzero(A == 0.0)
    if len(zr):
        logits = np.einsum("ij,ij->i", Hx[zr].astype(np.float64),
                           Hx[zc].astype(np.float64))
        Bv = np.maximum(1.0 / (1.0 + np.exp(-logits)), 0.1)
        np.subtract.at(pre, zr, (Bv * d[zc])[:, None] * HWs64[zc])
    out = d[:, None] * pre
    out = np.where(out >= 0, out, SLOPE * out).astype(np.float32)
    return out, A1
